# revision 7
# baseline (speedup 1.0000x reference)
"""MoME (multi-gate mixture-of-experts) Trainium2 kernel.

Data-parallel over 8 NeuronCores on the batch axis (2048 samples/core).
Per core:
  - embedding rows gathered from the (replicated) table via indirect DMA
    (one row per output partition-row per instruction; 256 instructions),
  - x transposed on the PE into feature-major layout [525, 2048],
  - per-task gate softmax (batch-major, free-dim softmax over E=8),
  - 8 experts: layer0 (f32r matmuls, BN folded into weights, relu via ACT
    with per-partition bias), layer1 emitted batch-major with the bias added
    through a K=1 ones-matmul, then the gate-combine fused into the relu:
    relu(z)*g == relu(z*g) for g>0, with g the per-partition gate scalar,
  - fea accumulated over experts on the DVE, transposed back feature-major,
  - per-task tower MLP + sigmoid.

BatchNorm (eval mode) is folded into weights/biases on the host; this is
layout preprocessing, all model arithmetic runs on device.
"""
import sys
sys.path.insert(0, "/opt/trn_rl_repo")
import numpy as np

import concourse.bass as bass
import concourse.mybir as mybir
import concourse.tile as tile
from concourse.bass_utils import run_bass_kernel_spmd
from concourse.masks import make_identity

f32 = mybir.dt.float32
f32r = mybir.dt.float32r
i32 = mybir.dt.int32
AF = mybir.ActivationFunctionType
OP = mybir.AluOpType

B = 16384; NF = 16; VOCAB = 100000; ED = 32; NNUM = 13
E = 8; T = 2; H0 = 512; H1 = 256; TH = 128
D0 = NF * ED + NNUM  # 525
EPS = 1e-5
NC = 8               # cores
BL = B // NC         # 2048 samples per core
NTAU = BL // 128     # 16 batch tiles per core
NROW = NF * VOCAB    # 1.6M table rows


# ---------------------------------------------------------------- wait split
def _split_excess_waits(nc, limit=1):
    """walrus CoreV3 codegen rejects >1 sync wait on several instruction
    lowerings (fused fp32 LW matmuls, drains).  Move excess waits onto
    preceding single-wait InstEventSemaphore carriers on the same engine."""
    counter = [0]
    for f in nc.m.functions:
        for b in f.blocks:
            out = []
            changed = False
            for ins in b.instructions:
                si = ins.sync_info
                waits = list(si.on_wait) if si and si.on_wait else []
                if len(waits) > limit:
                    changed = True
                    for wv in waits[:-limit]:
                        counter[0] += 1
                        nd = mybir.InstEventSemaphore(
                            name=f"wsplit-{counter[0]}", ins=[], outs=[])
                        nd.engine = ins.engine
                        nd.sync_info = mybir.SyncInfo(on_wait=[wv], on_update=[])
                        out.append(nd)
                    ins.sync_info = mybir.SyncInfo(
                        on_wait=waits[-limit:], on_update=list(si.on_update or []))
                out.append(ins)
            if changed:
                b.instructions = out


# ---------------------------------------------------------------- device code
def _build_module():
    nc = bass.Bass()
    dp = nc.declare_dram_parameter

    table = dp("table", [NROW, ED], f32, isOutput=False)
    idx = dp("idx", [128, NTAU * NF], i32, isOutput=False)
    numT = dp("numT", [NNUM, BL], f32, isOutput=False)
    w0a_d = dp("w0a", [E, 128, 4 * H0], f32, isOutput=False)     # [e][dp, kc*512+mc*128+m]
    w0b_d = dp("w0b", [E, NNUM, H0], f32, isOutput=False)        # tail K rows
    b0c_d = dp("b0c", [E, 128, 4], f32, isOutput=False)          # bias col per mc
    w1a_d = dp("w1a", [E, 128, 4 * H1], f32, isOutput=False)     # [e][rows, kc*256+n]
    b1r_d = dp("b1r", [E, H1], f32, isOutput=False)
    wg2a_d = dp("wg2a", [128, 4 * T * E], f32, isOutput=False)   # [dp, kc*16+te]
    wg2b_d = dp("wg2b", [NNUM, T * E], f32, isOutput=False)
    bgr_d = dp("bgr", [1, T * E], f32, isOutput=False)
    wt1_d = dp("wt1", [T, 128, 2 * TH], f32, isOutput=False)     # [t][rows, kc*128+m]
    bt1_d = dp("bt1", [T, TH, 1], f32, isOutput=False)
    wt2_d = dp("wt2", [T, TH, 1], f32, isOutput=False)
    bt2_d = dp("bt2", [1, T], f32, isOutput=False)
    ones_d = dp("onesr", [1, 128], f32, isOutput=False)
    out_d = dp("out", [T, BL], f32, isOutput=True)

    with tile.TileContext(nc) as tc:
        with (
            tc.tile_pool(name="const", bufs=1) as cpool,
            tc.tile_pool(name="sb", bufs=1) as pool,
            tc.tile_pool(name="wpool", bufs=2) as wpool,
            tc.tile_pool(name="xg", bufs=3) as xgpool,
            tc.tile_pool(name="tmp", bufs=4) as tpool,
            tc.tile_pool(name="ps", bufs=2, space="PSUM") as pspool,
            tc.tile_pool(name="psmm", bufs=3, space="PSUM") as psmm,
            tc.tile_pool(name="psz", bufs=2, space="PSUM") as psz,
        ):
            # ---- constants / small inputs
            ident = cpool.tile([128, 128], f32)
            make_identity(nc, ident[:])
            ones = cpool.tile([1, 128], f32r)
            nc.sync.dma_start(out=ones[:], in_=ones_d[:].bitcast(f32r))
            idx_sb = cpool.tile([128, NTAU * NF], i32)
            nc.sync.dma_start(out=idx_sb[:], in_=idx[:])
            numT_sb = cpool.tile([NNUM, BL], f32r)
            nc.sync.dma_start(out=numT_sb[:], in_=numT[:].bitcast(f32r))
            wg2a_sb = cpool.tile([128, 4 * T * E], f32r)
            nc.sync.dma_start(out=wg2a_sb[:], in_=wg2a_d[:].bitcast(f32r))
            wg2b_sb = cpool.tile([NNUM, T * E], f32r)
            nc.sync.dma_start(out=wg2b_sb[:], in_=wg2b_d[:].bitcast(f32r))
            bgr_sb = cpool.tile([1, T * E], f32r)
            nc.sync.dma_start(out=bgr_sb[:], in_=bgr_d[:].bitcast(f32r))
            wt1_sb = [cpool.tile([128, 2 * TH], f32r, tag=f"wt1_{t}", name=f"wt1_{t}") for t in range(T)]
            bt1_sb = [cpool.tile([TH, 1], f32, tag=f"bt1_{t}", name=f"bt1_{t}") for t in range(T)]
            wt2_sb = [cpool.tile([TH, 1], f32r, tag=f"wt2_{t}", name=f"wt2_{t}") for t in range(T)]
            bt2_sb = cpool.tile([1, T], f32)
            for t in range(T):
                nc.sync.dma_start(out=wt1_sb[t][:], in_=wt1_d[t].bitcast(f32r))
                nc.sync.dma_start(out=bt1_sb[t][:], in_=bt1_d[t])
                nc.sync.dma_start(out=wt2_sb[t][:], in_=wt2_d[t].bitcast(f32r))
            nc.sync.dma_start(out=bt2_sb[:], in_=bt2_d[:])

            # ---- persistent activations
            xT = [cpool.tile([128, BL], f32r, tag=f"xT{kc}", name=f"xT{kc}") for kc in range(4)]
            gn = [cpool.tile([128, T * E], f32, tag=f"gn{tau}", name=f"gn{tau}") for tau in range(NTAU)]
            fea = [[cpool.tile([128, H1], f32, tag=f"fea{t}_{tau}", name=f"fea{t}_{tau}")
                    for tau in range(NTAU)] for t in range(T)]
            h0T = [cpool.tile([128, BL], f32r, tag=f"h0T{kc}", name=f"h0T{kc}") for kc in range(4)]
            out_sb = [cpool.tile([1, BL], f32, tag=f"out{t}", name=f"out{t}") for t in range(T)]

            # ---- phase 1: gather + transpose + gates, per batch tile tau
            for tau in range(NTAU):
                xg = xgpool.tile([128, NF * ED], f32r, tag="xg")
                for ff in range(NF):
                    g = tau * NF + ff
                    nc.gpsimd.indirect_dma_start(
                        out=xg[:, ff * ED:(ff + 1) * ED],
                        out_offset=None,
                        in_=table[:].bitcast(f32r),
                        in_offset=bass.IndirectOffsetOnAxis(
                            ap=idx_sb[:, g:g + 1], axis=0),
                    )
                for kc in range(4):
                    tp = pspool.tile([128, 128], f32, space="PSUM", tag="tr")
                    nc.tensor.transpose(
                        out=tp[:], in_=xg[:, kc * 128:(kc + 1) * 128].bitcast(f32),
                        identity=ident[:])
                    nc.vector.tensor_copy(
                        out=xT[kc][:, tau * 128:(tau + 1) * 128], in_=tp[:])

                # gates (batch-major): logits [128, 16]
                lg = pspool.tile([128, T * E], f32, space="PSUM", tag="lg", bufs=1)
                nc.tensor.matmul(out=lg[:], lhsT=ones[:], rhs=bgr_sb[:],
                                 start=True, stop=False)
                for kc in range(4):
                    nc.tensor.matmul(
                        out=lg[:],
                        lhsT=xT[kc][:, tau * 128:(tau + 1) * 128],
                        rhs=wg2a_sb[:, kc * T * E:(kc + 1) * T * E],
                        start=False, stop=False)
                nc.tensor.matmul(
                    out=lg[:], lhsT=numT_sb[:, tau * 128:(tau + 1) * 128],
                    rhs=wg2b_sb[:], start=False, stop=True)
                ge = tpool.tile([128, T * E], f32, tag="ge")
                nc.scalar.activation(out=ge[:], in_=lg[:], func=AF.Exp)
                gs = tpool.tile([128, T], f32, tag="gs")
                nc.vector.reduce_sum(
                    out=gs[:], in_=ge[:].rearrange("p (t e) -> p t e", e=E),
                    axis=mybir.AxisListType.X)
                gr = tpool.tile([128, T], f32, tag="gr")
                nc.vector.reciprocal(out=gr[:], in_=gs[:])
                for t in range(T):
                    nc.vector.tensor_tensor(
                        out=gn[tau][:, t * E:(t + 1) * E],
                        in0=ge[:, t * E:(t + 1) * E],
                        in1=gr[:, t:t + 1].to_broadcast([128, E]),
                        op=OP.mult)

            # ---- phase 2: experts
            for e in range(E):
                w0a = wpool.tile([128, 4 * H0], f32r, tag="w0a")
                nc.sync.dma_start(out=w0a[:], in_=w0a_d[e].bitcast(f32r))
                w0b = wpool.tile([NNUM, H0], f32r, tag="w0b")
                nc.sync.dma_start(out=w0b[:], in_=w0b_d[e].bitcast(f32r))
                b0c = wpool.tile([128, 4], f32, tag="b0c")
                nc.sync.dma_start(out=b0c[:], in_=b0c_d[e])
                w1a = wpool.tile([128, 4 * H1], f32r, tag="w1a")
                nc.sync.dma_start(out=w1a[:], in_=w1a_d[e].bitcast(f32r))
                b1r = wpool.tile([1, H1], f32r, tag="b1r")
                nc.sync.dma_start(out=b1r[:], in_=b1r_d[e:e + 1].bitcast(f32r))

                # layer0: h0T[mc][:, n*512:+512] = relu(x @ W0eff + b0eff)
                for mc in range(4):
                    for n in range(4):
                        ph = psmm.tile([128, 512], f32, space="PSUM", tag="h0")
                        for kc in range(4):
                            nc.tensor.matmul(
                                out=ph[:],
                                lhsT=w0a[:, kc * H0 + mc * 128:kc * H0 + (mc + 1) * 128],
                                rhs=xT[kc][:, n * 512:(n + 1) * 512],
                                start=(kc == 0), stop=False)
                        nc.tensor.matmul(
                            out=ph[:], lhsT=w0b[:, mc * 128:(mc + 1) * 128],
                            rhs=numT_sb[:, n * 512:(n + 1) * 512],
                            start=False, stop=True)
                        nc.scalar.activation(
                            out=h0T[mc][:, n * 512:(n + 1) * 512], in_=ph[:],
                            func=AF.Relu, bias=b0c[:, mc:mc + 1], scale=1.0)

                # layer1 (batch-major) + gated relu + accumulate
                for tau in range(NTAU):
                    pz = psz.tile([128, H1], f32, space="PSUM", tag="z1")
                    nc.tensor.matmul(out=pz[:], lhsT=ones[:], rhs=b1r[:],
                                     start=True, stop=False)
                    for kc in range(4):
                        nc.tensor.matmul(
                            out=pz[:],
                            lhsT=h0T[kc][:, tau * 128:(tau + 1) * 128],
                            rhs=w1a[:, kc * H1:(kc + 1) * H1],
                            start=False, stop=(kc == 3))
                    for t in range(T):
                        if e == 0:
                            nc.scalar.activation(
                                out=fea[t][tau][:], in_=pz[:], func=AF.Relu,
                                scale=gn[tau][:, t * E + e:t * E + e + 1])
                        else:
                            tm = tpool.tile([128, H1], f32, tag="tm")
                            nc.scalar.activation(
                                out=tm[:], in_=pz[:], func=AF.Relu,
                                scale=gn[tau][:, t * E + e:t * E + e + 1])
                            nc.vector.tensor_tensor(
                                out=fea[t][tau][:], in0=fea[t][tau][:],
                                in1=tm[:], op=OP.add)

            # ---- phase 3: towers
            for tau in range(NTAU):
                for t in range(T):
                    feaT = tpool.tile([128, H1], f32r, tag="feaT")
                    for kc in range(2):
                        tp = pspool.tile([128, 128], f32, space="PSUM", tag="tr")
                        nc.tensor.transpose(
                            out=tp[:], in_=fea[t][tau][:, kc * 128:(kc + 1) * 128],
                            identity=ident[:])
                        nc.vector.tensor_copy(
                            out=feaT[:, kc * 128:(kc + 1) * 128], in_=tp[:])
                    p1 = pspool.tile([128, TH], f32, space="PSUM", tag="tr")
                    for kc in range(2):
                        nc.tensor.matmul(
                            out=p1[:], lhsT=wt1_sb[t][:, kc * TH:(kc + 1) * TH],
                            rhs=feaT[:, kc * 128:(kc + 1) * 128],
                            start=(kc == 0), stop=(kc == 1))
                    t1 = tpool.tile([128, TH], f32r, tag="t1")
                    nc.scalar.activation(out=t1[:], in_=p1[:], func=AF.Relu,
                                         bias=bt1_sb[t][:, 0:1], scale=1.0)
                    p2 = pspool.tile([1, TH], f32, space="PSUM", tag="lg", bufs=1)
                    nc.tensor.matmul(out=p2[:], lhsT=wt2_sb[t][:], rhs=t1[:],
                                     start=True, stop=True)
                    nc.scalar.activation(
                        out=out_sb[t][0:1, tau * 128:(tau + 1) * 128], in_=p2[:],
                        func=AF.Sigmoid, bias=bt2_sb[0:1, t:t + 1], scale=1.0)

            for t in range(T):
                nc.sync.dma_start(out=out_d[t:t + 1, :], in_=out_sb[t][:])

    _split_excess_waits(nc)
    return nc


_NC_CACHE = None


def _get_module():
    global _NC_CACHE
    if _NC_CACHE is None:
        _NC_CACHE = _build_module()
    return _NC_CACHE


# ---------------------------------------------------------------- host side
def _prep_inputs(inputs):
    f = np.float32
    cat = np.asarray(inputs["categorical_x"]).astype(np.int64)
    flat = (cat + (np.arange(NF, dtype=np.int64) * VOCAB)[None, :]).astype(np.int32)
    # idx[c][p, tau*NF + ff] = flat[c*BL + tau*128 + p, ff]
    idx_all = flat.reshape(NC, NTAU, 128, NF).transpose(0, 2, 1, 3) \
                  .reshape(NC, 128, NTAU * NF).copy()
    num = np.asarray(inputs["numerical_x"], f)
    numT_all = num.reshape(NC, BL, NNUM).transpose(0, 2, 1).copy()  # [c, 13, BL]

    s0 = (np.asarray(inputs["bn0_gamma"], f)
          / np.sqrt(np.asarray(inputs["bn0_var"], f) + EPS))
    W0e = (np.asarray(inputs["W0"], f) * s0[None, None, :]).astype(f)
    b0e = ((np.asarray(inputs["b0"], f) - np.asarray(inputs["bn0_mean"], f)[None, :])
           * s0[None, :] + np.asarray(inputs["bn0_beta"], f)[None, :]).astype(f)
    w0a = W0e[:, :512, :].reshape(E, 4, 128, H0).transpose(0, 2, 1, 3) \
                         .reshape(E, 128, 4 * H0).copy()
    w0b = W0e[:, 512:, :].copy()                       # [E, 13, 512]
    b0c = b0e.reshape(E, 4, 128).transpose(0, 2, 1).copy()  # [E, 128, 4]

    s1 = (np.asarray(inputs["bn1_gamma"], f)
          / np.sqrt(np.asarray(inputs["bn1_var"], f) + EPS))
    W1e = (np.asarray(inputs["W1"], f) * s1[None, None, :]).astype(f)
    b1e = ((np.asarray(inputs["b1"], f) - np.asarray(inputs["bn1_mean"], f)[None, :])
           * s1[None, :] + np.asarray(inputs["bn1_beta"], f)[None, :]).astype(f)
    w1a = W1e.reshape(E, 4, 128, H1).transpose(0, 2, 1, 3) \
             .reshape(E, 128, 4 * H1).copy()
    b1r = b1e.copy()                                   # [E, 256]

    Wg = np.asarray(inputs["Wg"], f)                   # [T, 525, E]
    Wg2 = Wg.transpose(1, 0, 2).reshape(D0, T * E)
    wg2a = Wg2[:512].reshape(4, 128, T * E).transpose(1, 0, 2) \
                    .reshape(128, 4 * T * E).copy()
    wg2b = Wg2[512:].copy()                            # [13, 16]
    bgr = np.asarray(inputs["bg"], f).reshape(1, T * E).copy()

    st = (np.asarray(inputs["tbn_gamma"], f)
          / np.sqrt(np.asarray(inputs["tbn_var"], f) + EPS))  # [T, TH]
    Wt1e = (np.asarray(inputs["Wt1"], f) * st[:, None, :]).astype(f)  # [T,256,128]
    bt1e = ((np.asarray(inputs["bt1"], f) - np.asarray(inputs["tbn_mean"], f)) * st
            + np.asarray(inputs["tbn_beta"], f)).astype(f)            # [T, 128]
    wt1 = Wt1e.reshape(T, 2, 128, TH).transpose(0, 2, 1, 3) \
              .reshape(T, 128, 2 * TH).copy()
    bt1 = bt1e[:, :, None].copy()                      # [T, 128, 1]
    wt2 = np.asarray(inputs["Wt2"], f).copy()          # [T, 128, 1]
    bt2 = np.asarray(inputs["bt2"], f).reshape(1, T).copy()

    table = np.ascontiguousarray(np.asarray(inputs["emb_table"], f))

    shared = dict(table=table, w0a=w0a, w0b=w0b, b0c=b0c, w1a=w1a, b1r=b1r,
                  wg2a=wg2a, wg2b=wg2b, bgr=bgr, wt1=wt1, bt1=bt1, wt2=wt2,
                  bt2=bt2, onesr=np.ones((1, 128), np.float32))
    in_maps = []
    for c in range(NC):
        m = dict(shared)
        m["idx"] = idx_all[c]
        m["numT"] = numT_all[c]
        in_maps.append(m)
    return in_maps


def kernel(**inputs) -> np.ndarray:
    nc = _get_module()
    in_maps = _prep_inputs(inputs)
    results = run_bass_kernel_spmd(nc, in_maps, list(range(NC))).results
    out = np.empty((T, B), np.float32)
    for c in range(NC):
        out[:, c * BL:(c + 1) * BL] = results[c]["out"]
    return out
